# revision 10
# baseline (speedup 1.0000x reference)
"""Trainium2 Bass kernel for the emoji-box decoder problem (v2).

Math: softmax(-d2) over emoji pixels is separable (rows x cols).  Per
core (64 canvas rows x 256 cols x 3ch of one picture):

  E = exp(-D2),  D2 built by ONE PE matmul with lhsT=[1, p-32, (p-32)^2]
  and rhs rows [u^2; 2*beta*u; beta^2] (u = gamma - n), giving
    E[:, 0:128]   = erT[i, (h,r)]  (row kernel, transposed, h-duplicated)
    E[:, 128:384] = ecT[j, c]      (col kernel, transposed)
  T1_ch[j, r] = sum_i img[ch,i,j] * erT[i,r]          (3 matmuls)
  U[(ch,r), c] = sum_j T1[j,(ch,r)] * ecT[j,c]        (2 matmuls)
  res = U * srow[(ch,r)] * SCOL[c] + Qv               (2 DVE ops/piece)
  with srow = rzr*rowin (transposed via PE), SCOL = ones (x) (rzc*colin),
  Qv = valid - rowin (x) colin (PE outer products).

No max-shift in the softmax: in-box distances are <= ~16 so exp(-d2)
stays in range; fully-underflowed rows are saved by a 1e-30 clamp on
the denominator and are masked anyway.

The final blend + output DMAs run OUTSIDE the TileContext so the fixed
NRT postamble (a ~6us per-engine semaphore-clear storm, longest on the
Tensor engine) overlaps the output-DMA tail instead of following it.

Sharding: 8 cores = 2 pictures x 4 row-blocks of 64 canvas rows; images
replicated; xmeta = [X row (19), r0].
"""

import sys

import numpy as np

if "/opt/trn_rl_repo" not in sys.path:
    sys.path.insert(0, "/opt/trn_rl_repo")

import concourse.bacc as bacc
import concourse.bass as bass
import concourse.mybir as mybir
import concourse.tile as tile
from concourse.bass_utils import run_bass_kernel_spmd


def _ensure_ntff_hook():
    """The image's antenv package lacks axon_hooks, so trn_boot's NTFF
    profile hook install degrades silently and run_bass_kernel_spmd
    crashes on `from antenv.axon_hooks import ...` when trace=True.
    Provide the module and install the ctypes hook ourselves."""
    import types

    try:
        from antenv.axon_hooks import get_axon_ntff_profile_hook  # noqa: F401

        return
    except ImportError:
        pass
    mod = types.ModuleType("antenv.axon_hooks")
    _hook = [None]
    mod.set_axon_ntff_profile_hook = lambda h: _hook.__setitem__(0, h)
    mod.get_axon_ntff_profile_hook = lambda: _hook[0]
    try:
        import antenv

        sys.modules["antenv.axon_hooks"] = mod
        antenv.axon_hooks = mod
        from trn_agent_boot.trn_boot import _ntff_profile_via_ctypes

        hook = _ntff_profile_via_ctypes("/opt/axon/libaxon_pjrt.so")
        if hook is not None:
            mod.set_axon_ntff_profile_hook(hook)
    except Exception:
        pass


_ensure_ntff_hook()

F32 = mybir.dt.float32
I32 = mybir.dt.int32
U32 = mybir.dt.uint32
AF = mybir.ActivationFunctionType
OP = mybir.AluOpType
AX = mybir.AxisListType

MAGIC = 8388608.0  # 2**23; x + MAGIC - MAGIC == rint(x) for 0 <= x < 2**22

N_CORES = 8
H = 256
S = 64
N_IMG = 14
RB = 64  # canvas rows per core


def build_nc():
    nc = bacc.Bacc("TRN2", target_bir_lowering=False, debug=False)

    xmeta_d = nc.dram_tensor("xmeta", [1, 20], F32, kind="ExternalInput")
    images_d = nc.dram_tensor("images", [N_IMG, 4, S, S], F32, kind="ExternalInput")
    out_d = nc.dram_tensor("out", [3, RB, H], F32, kind="ExternalOutput")

    # ---- raw allocations (persist across the TileContext boundary) ----
    sb = lambda name, shape, dt=F32: nc.alloc_sbuf_tensor(name, shape, dt)
    xrow = sb("xrow", [1, 20])
    iota256f = sb("iota256f", [1, 256])
    iotadupf = sb("iotadupf", [1, 128])      # [0..63, 0..63]
    lhsT3 = sb("lhsT3", [1, 192])            # [1s | p-32 | (p-32)^2] rows, all p0
    ones_col64 = sb("ones_col64", [64, 1])
    ones128_row = sb("ones128_row", [1, 128])
    one11 = sb("one11", [1, 1])
    blk = sb("blk", [1, 12])                 # scalar block (all p0)
    cs = sb("cs", [1, 4])
    box2 = sb("box2", [1, 2])
    beta2 = sb("beta2", [1, 2])
    v10 = sb("v10", [1, 10])
    tmp11 = sb("tmp11", [1, 2])
    u384 = sb("u384", [1, 384])
    rhs3 = sb("rhs3", [1, 3, 384])           # three rank-1 rows, all at p0
    E = sb("E", [64, 384])
    wimg = sb("wimg", [64, 3, 64])
    T1sb = sb("T1sb", [64, 192])
    mrow = sb("mrow", [1, 128])
    mcol = sb("mcol", [1, 256])
    mtmp = sb("mtmp", [1, 512])
    zsb = sb("zsb", [1, 384])
    rz = sb("rz", [1, 384])
    srow_dup = sb("srow_dup", [1, 128])
    scol = sb("scol", [1, 256])
    ssb = sb("ssb", [128, 1])
    vrow = sb("vrow", [1, 128])
    negcol = sb("negcol", [1, 256])
    ones256_row = sb("ones256_row", [1, 256])
    mx8 = sb("mx8", [1, 8])
    idx8 = sb("idx8", [1, 8], U32)
    scol_sb = sb("scol_sb", [128, 256])
    res_ab = sb("res_ab", [128, 256])
    res_c = sb("res_c", [64, 256])
    warm1 = sb("warm1", [1, 1])
    warm2 = sb("warm2", [1, 1])

    ps = lambda name, shape: nc.alloc_psum_tensor(name, shape)
    D2ps = ps("D2ps", [64, 384])
    zps = ps("zps", [128, 512])              # z row + s_stacked column share a bank
    T1ps = ps("T1ps", [64, 192])
    Uab_ps = ps("Uab_ps", [128, 256])
    Uc_ps = ps("Uc_ps", [64, 256])
    scol_ps = ps("scol_ps", [128, 256])
    Qv_ps = ps("Qv_ps", [128, 256])

    with tile.TileContext(nc) as tc:  # noqa: F841
        # ---- warm the scalar-engine Exp table early (overlaps input DMA)
        nc.gpsimd.memset(warm1[:], 0.0)
        nc.scalar.activation(warm2[:], warm1[:], AF.Exp)

        # ---- input DMA first
        nc.sync.dma_start(xrow[:], xmeta_d[:])

        # ---- constants (no data deps); everything lives at partition 0
        it_i = sb("it_i", [1, 256], I32)
        nc.gpsimd.iota(it_i[0:1, :], pattern=[[1, 256]], base=0, channel_multiplier=0)
        nc.vector.tensor_copy(iota256f[0:1, :], it_i[0:1, :])
        itd_i = sb("itd_i", [1, 128], I32)
        nc.gpsimd.iota(itd_i[0:1, :], pattern=[[0, 2], [1, 64]], base=0, channel_multiplier=0)
        nc.vector.tensor_copy(iotadupf[0:1, :], itd_i[0:1, :])
        l3_i = sb("l3_i", [1, 64], I32)
        nc.gpsimd.iota(l3_i[0:1, :], pattern=[[1, 64]], base=-32, channel_multiplier=0)
        nc.vector.memset(lhsT3[0:1, 0:64], 1.0)
        nc.vector.tensor_copy(lhsT3[0:1, 64:128], l3_i[0:1, :])
        nc.vector.tensor_tensor(
            lhsT3[0:1, 128:192], lhsT3[0:1, 64:128], lhsT3[0:1, 64:128], OP.mult
        )
        nc.gpsimd.memset(ones_col64[:], 1.0)
        nc.vector.memset(ones128_row[:], 1.0)
        nc.vector.memset(one11[:], 1.0)
        nc.vector.memset(ones256_row[:], 1.0)

        # ---- emoji index: top-1 via Max8 + MaxIndex, straight off xrow
        nc.vector.max(mx8[:], xrow[0:1, 5:19])
        nc.vector.max_index(idx8[:], mx8[:], xrow[0:1, 5:19])
        with nc.gpsimd.register("ridx") as ridx:
            nc.gpsimd.reg_load(ridx, idx8[0:1, 0:1])
            off = nc.gpsimd.snap(ridx)
            nc.gpsimd.dma_start(
                wimg[:],
                images_d[bass.ds(off, 1), 0:3, :, :].squeeze(0).transpose([1, 0, 2]),
            )

        # ---- p0 scalar chain -> blk
        # cs = rint(256*X[0:4])
        nc.vector.tensor_scalar(cs[:], xrow[0:1, 0:4], 256.0, MAGIC, OP.mult, OP.add)
        nc.vector.tensor_scalar(cs[:], cs[:], MAGIC, None, OP.subtract)
        # box2 = [x2-x1, y2-y1]; beta2 = box2/64
        nc.vector.tensor_tensor(box2[:], cs[0:1, 1:4:2], cs[0:1, 0:3:2], OP.subtract)
        nc.vector.tensor_scalar(beta2[:], box2[:], 1.0 / 64.0, None, OP.mult)
        # x1r = x1 - r0
        nc.vector.tensor_tensor(blk[0:1, 6:7], cs[0:1, 0:1], xrow[0:1, 19:20], OP.subtract)
        # gamma_r = x1r + 32*beta_r ; gamma_c = y1 + 32*beta_c
        nc.vector.scalar_tensor_tensor(
            blk[0:1, 0:1], beta2[0:1, 0:1], 32.0, blk[0:1, 6:7], OP.mult, OP.add
        )
        nc.vector.scalar_tensor_tensor(
            blk[0:1, 1:2], beta2[0:1, 1:2], 32.0, cs[0:1, 2:3], OP.mult, OP.add
        )
        # 2*beta, beta^2
        nc.vector.tensor_scalar(blk[0:1, 2:4], beta2[:], 2.0, None, OP.mult)
        nc.vector.tensor_tensor(blk[0:1, 4:6], beta2[:], beta2[:], OP.mult)

        # valid = AND of 10 conditions (min)
        nc.vector.tensor_scalar(v10[0:1, 0:4], cs[:], 0.0, None, OP.is_ge)
        nc.vector.tensor_scalar(v10[0:1, 4:8], cs[:], 256.0, None, OP.is_le)
        nc.vector.tensor_tensor(v10[0:1, 8:10], cs[0:1, 1:4:2], cs[0:1, 0:3:2], OP.is_gt)
        nc.vector.tensor_reduce(blk[0:1, 10:11], v10[:], AX.X, OP.min)
        # x2r poisoned: x2 - r0 + (valid-1)*1e9
        nc.vector.tensor_scalar(tmp11[0:1, 0:1], blk[0:1, 10:11], 1e9, -1e9, OP.mult, OP.add)
        nc.vector.tensor_tensor(tmp11[0:1, 1:2], cs[0:1, 1:2], xrow[0:1, 19:20], OP.subtract)
        nc.vector.tensor_tensor(blk[0:1, 7:8], tmp11[0:1, 1:2], tmp11[0:1, 0:1], OP.add)
        # y1, y2
        nc.vector.tensor_copy(blk[0:1, 8:10], cs[0:1, 2:4])

        # ---- rhs3 rows for the D2 matmuls (all at p0)
        # row0: u^2 with u = gamma - n
        nc.vector.tensor_scalar(
            u384[0:1, 0:128], iotadupf[0:1, :], -1.0, blk[0:1, 0:1], OP.mult, OP.add
        )
        nc.vector.tensor_scalar(
            u384[0:1, 128:384], iota256f[0:1, :], -1.0, blk[0:1, 1:2], OP.mult, OP.add
        )
        nc.vector.tensor_tensor(rhs3[0:1, 0, :], u384[0:1, :], u384[0:1, :], OP.mult)
        # row1: 2*beta*u
        nc.vector.tensor_scalar(
            rhs3[0:1, 1, 0:128], u384[0:1, 0:128], blk[0:1, 2:3], None, OP.mult
        )
        nc.vector.tensor_scalar(
            rhs3[0:1, 1, 128:384], u384[0:1, 128:384], blk[0:1, 3:4], None, OP.mult
        )
        # row2: beta^2 replicated
        nc.vector.tensor_scalar(
            rhs3[0:1, 2, 0:128], ones128_row[:], blk[0:1, 4:5], None, OP.mult
        )
        nc.vector.tensor_scalar(
            rhs3[0:1, 2, 128:384], ones256_row[:], blk[0:1, 5:6], None, OP.mult
        )

        # ---- D2 = sum of three rank-1 matmuls ; E = exp(-D2)
        nc.tensor.matmul(
            D2ps[:, :], lhsT3[0:1, 0:64], rhs3[0:1, 0, :], start=True, stop=False
        )
        nc.tensor.matmul(
            D2ps[:, :], lhsT3[0:1, 64:128], rhs3[0:1, 1, :], start=False, stop=False
        )
        nc.tensor.matmul(
            D2ps[:, :], lhsT3[0:1, 128:192], rhs3[0:1, 2, :], start=False, stop=True
        )
        nc.scalar.activation(E[:], D2ps[:, :], AF.Exp, scale=-1.0)

        # ---- denominators: z = ones_col64.T @ E (row sums across partitions)
        nc.tensor.matmul(zps[0:1, 0:384], ones_col64[:], E[:])
        nc.vector.tensor_scalar(zsb[:], zps[0:1, 0:384], 1e-30, None, OP.max)
        nc.vector.reciprocal(rz[:], zsb[:])

        # ---- masks @p0
        nc.vector.tensor_scalar(mtmp[0:1, 0:128], iotadupf[0:1, :], blk[0:1, 6:7], None, OP.is_ge)
        nc.vector.scalar_tensor_tensor(
            mrow[:], iotadupf[0:1, :], blk[0:1, 7:8], mtmp[0:1, 0:128], OP.is_lt, OP.mult
        )
        nc.vector.tensor_scalar(mtmp[0:1, 256:512], iota256f[0:1, :], blk[0:1, 8:9], None, OP.is_ge)
        nc.vector.scalar_tensor_tensor(
            mcol[:], iota256f[0:1, :], blk[0:1, 9:10], mtmp[0:1, 256:512], OP.is_lt, OP.mult
        )
        nc.vector.tensor_tensor(srow_dup[:], rz[0:1, 0:128], mrow[:], OP.mult)
        nc.vector.tensor_tensor(scol[:], rz[0:1, 128:384], mcol[:], OP.mult)

        # ---- s_stacked = transpose(srow_dup) via PE ; SCOL = ones (x) scol
        nc.tensor.matmul(zps[:, 384:385], srow_dup[:], one11[:])
        nc.vector.tensor_copy(ssb[:], zps[:, 384:385])
        nc.tensor.matmul(scol_ps[:, :], ones128_row[:], scol[:])
        nc.scalar.copy(scol_sb[:], scol_ps[:, :])

        # ---- Qv = valid (x) ones - rowin (x) colin  (two rank-1 matmuls,
        # reusing the mask rows; mrow is valid-poisoned already)
        nc.vector.tensor_scalar(vrow[:], ones128_row[:], blk[0:1, 10:11], None, OP.mult)
        nc.vector.tensor_scalar(negcol[:], mcol[:], -1.0, None, OP.mult)
        nc.tensor.matmul(Qv_ps[:, :], vrow[:], ones256_row[:], start=True, stop=False)
        nc.tensor.matmul(Qv_ps[:, :], mrow[:], negcol[:], start=False, stop=True)

        # ---- T1 matmuls (need wimg + E)
        nc.tensor.matmul(T1ps[:, 0:64], wimg[:, 0, :], E[:, 0:64])
        nc.tensor.matmul(T1ps[:, 64:128], wimg[:, 1, :], E[:, 64:128])
        nc.tensor.matmul(T1ps[:, 128:192], wimg[:, 2, :], E[:, 0:64])
        nc.scalar.copy(T1sb[:], T1ps[:, :])

        # ---- U matmuls
        nc.tensor.matmul(Uab_ps[:, :], T1sb[:, 0:128], E[:, 128:384])
        nc.tensor.matmul(Uc_ps[:, :], T1sb[:, 128:192], E[:, 128:384])

    # ---- post-tile: blend + output DMAs (overlaps the NRT postamble storm)
    semA = nc.alloc_semaphore("postA")
    semB = nc.alloc_semaphore("postB")
    semD = nc.alloc_semaphore("postD")
    semE = nc.alloc_semaphore("postE")

    nc.vector.scalar_tensor_tensor(
        res_ab[:], Uab_ps[:, :], ssb[:, 0:1], scol_sb[:, :], OP.mult, OP.mult
    )
    nc.vector.drain()
    nc.vector.tensor_tensor(res_ab[:], res_ab[:], Qv_ps[:, :], OP.add).then_inc(semA)
    nc.vector.scalar_tensor_tensor(
        res_c[:], Uc_ps[:, :], ssb[0:64, 0:1], scol_sb[0:64, :], OP.mult, OP.mult
    )
    nc.vector.drain()
    nc.vector.tensor_tensor(res_c[:], res_c[:], Qv_ps[0:64, :], OP.add).then_inc(semB)

    nc.sync.wait_ge(semA, 1)
    nc.sync.dma_start(
        out_d[0:2, :, :].rearrange("a b c -> (a b) c"), res_ab[:]
    ).then_inc(semD, 16)
    nc.scalar.wait_ge(semB, 1)
    nc.scalar.dma_start(out_d[2, :, :], res_c[:]).then_inc(semE, 16)
    nc.sync.wait_ge(semD, 16)
    nc.scalar.wait_ge(semE, 16)

    nc.compile()
    return nc


_CACHE = {}


def get_nc():
    if "nc" not in _CACHE:
        _CACHE["nc"] = build_nc()
    return _CACHE["nc"]


def make_in_maps(X, images):
    X = np.ascontiguousarray(np.asarray(X, np.float32))
    images = np.ascontiguousarray(np.asarray(images, np.float32))
    in_maps = []
    for c in range(N_CORES):
        pic, rb = divmod(c, 4)
        xm = np.zeros((1, 20), np.float32)
        xm[0, :19] = X[pic, 0]
        xm[0, 19] = float(RB * rb)
        in_maps.append({"xmeta": xm, "images": images})
    return in_maps


def assemble(results):
    out = np.empty((2, 3, H, H), np.float32)
    for c in range(N_CORES):
        pic, rb = divmod(c, 4)
        out[pic, :, RB * rb : RB * (rb + 1), :] = results[c]["out"]
    return out


def _axon_reset():
    try:
        import ctypes

        import jax

        jax.devices()
        ctypes.CDLL("/opt/axon/libaxon_pjrt.so").axon_reset()
    except Exception:
        pass


def kernel(X, images):
    nc = get_nc()
    in_maps = make_in_maps(X, images)
    try:
        res = run_bass_kernel_spmd(nc, in_maps, list(range(N_CORES)))
    except Exception:
        # the axon terminal can be left in a bad state by earlier failed
        # runs (LoadExecutable errors); reset and retry once
        _axon_reset()
        res = run_bass_kernel_spmd(nc, in_maps, list(range(N_CORES)))
    return assemble(res.results)
